# revision 39
# baseline (speedup 1.0000x reference)
"""ALiBi causal attention block (QKV proj + attention + out proj) on 8 TRN2
NeuronCores, written in Bass/Tile. v3: all-SBUF, all-bf16, ALiBi tile skipping.

Sharding: batch(2) x head-group(4) -> 8 cores. Core c (b=c//4, g=c%4) runs
heads {(3-hl)*4+g : hl in 0..3} of its batch: slot hl on every core holds a
head from the same slope quartile, so the causal+ALiBi tile-skip pattern is
identical across cores (one SPMD program) and balanced. QKV projection and
attention are comm-free per core. A per-head 8-core AllToAll (bf16)
redistributes attention outputs from head-sharding to row-sharding; a zsel
0/1 blend drops the duplicate cross-batch shards. Each core then multiplies
its 512 output rows by the full Wo (bf16) and writes rows
[512g, 512(g+1)) of its batch.

v3 vs the DRAM-roundtrip baseline:
- q/k/v stay SBUF-resident between projection and attention (bf16); no
  per-head DRAM reloads in phase B.
- All GEMMs in bf16 (1 cycle/row on the PE); x transposed in fp32r and cast
  to bf16 on the PSUM->SBUF copy (ACT).
- ALiBi decays by ~e^-25 within B_SLOT tiles of the diagonal, so far
  below-diagonal score tiles are skipped: 121 of 160 tiles per core.
- Bias+mask tiles precomputed on host (bf16), DMA'd in; the gpsimd queue
  holds only collective triggers + half the bias-add STTs, so each head's
  AllToAll fires right after its outputs ship.
- Phase-B bias adds alternate DVE/Pool; exps on ACT; the PE stream is
  software-pipelined 2 deep so it never waits on the softmax chain.
- Transposes for chunk s4+1 emitted interleaved between chunk s4's
  projection chains.
"""

import math
from collections import deque

import numpy as np

import concourse.bass as bass
import concourse.mybir as mybir
import concourse.tile as tile
from concourse import bacc
from concourse.bass_utils import run_bass_kernel_spmd
from concourse.masks import make_identity

F32 = mybir.dt.float32
F32R = mybir.dt.float32r
BF16 = mybir.dt.bfloat16
AL = mybir.AluOpType
AF = mybir.ActivationFunctionType

HIDDEN = 2048
NUM_HEADS = 16
HEAD = 128
SEQ = 2048
BATCH = 2
N_CORES = 8
HL = 4                      # heads per core
QD = HL * HEAD              # 512 projected cols per core
SCALE = 1.0 / math.sqrt(HEAD)
NEG = -1.0e6
ST = SEQ // 128             # 16 seq tiles
S4 = SEQ // 512             # 4 coarse chunks
ET = HIDDEN // 128          # 16 contraction tiles
RQ = SEQ // 4               # 512 output rows per core
RT = RQ // 128              # 4

# Below-diagonal tiles kept per slot: keep jt >= 4*im - B_SLOT[hl]. Slot hl
# holds head (3-hl)*4+g, so slot 0 has the smallest slopes (keep everything)
# and slot 3 the largest (keep only 1 below-diagonal tile).
B_SLOT = [16, 9, 3, 1]


def head_of(g, hl):
    return (3 - hl) * 4 + g


def _slopes():
    if NUM_HEADS <= 8:
        return [1.0 / 2 ** k for k in range(NUM_HEADS)]
    return [1.0 / 2 ** (k / 2) for k in range(NUM_HEADS)]


def _r(ap):
    return ap.bitcast(F32R)


def build_nc(seq=SEQ):
    E = HIDDEN

    nc = bacc.Bacc("TRN2", target_bir_lowering=False, debug=False,
                   num_devices=N_CORES)

    x_d = nc.dram_tensor("x", [seq, E], F32, kind="ExternalInput").ap()
    wq_d = nc.dram_tensor("wq", [E, QD], BF16, kind="ExternalInput").ap()
    wk_d = nc.dram_tensor("wk", [E, QD], BF16, kind="ExternalInput").ap()
    wv_d = nc.dram_tensor("wv", [E, QD], BF16, kind="ExternalInput").ap()
    bq_d = nc.dram_tensor("bq", [QD], F32, kind="ExternalInput").ap()
    bk_d = nc.dram_tensor("bk", [QD], F32, kind="ExternalInput").ap()
    bv_d = nc.dram_tensor("bv", [QD], F32, kind="ExternalInput").ap()
    wo_d = nc.dram_tensor("wo", [E, E], BF16, kind="ExternalInput").ap()
    bo_d = nc.dram_tensor("bo", [E], F32, kind="ExternalInput").ap()
    # slot 3 (largest slopes): pre-exp bias tables (STT path)
    bjv_d = nc.dram_tensor("bjv", [128, S4 * ST], F32,
                           kind="ExternalInput").ap()
    bim_d = nc.dram_tensor("bim", [128, 512], BF16,
                           kind="ExternalInput").ap()
    bmask_d = nc.dram_tensor("bmask", [128, 4 * 512], BF16,
                             kind="ExternalInput").ap()
    # slots 0-2: factorized form. The per-column factor exp(-sl*(i-anchor))
    # cancels in the softmax normalization, so only the 0/1 causal pattern
    # (head-independent) is applied post-exp.
    bjv2_d = nc.dram_tensor("bjv2", [128, 3 * S4 * ST], F32,
                            kind="ExternalInput").ap()
    cau_d = nc.dram_tensor("cau", [128, 4 * 512], BF16,
                           kind="ExternalInput").ap()
    zsel_d = nc.dram_tensor("zsel", [128, 2], F32, kind="ExternalInput").ap()
    out_d = nc.dram_tensor("out", [RQ, E], F32, kind="ExternalOutput").ap()

    with tile.TileContext(nc) as tc:
        with (
            tc.tile_pool(name="const", bufs=1) as cpool,
            tc.tile_pool(name="persist", bufs=1) as pers,
            tc.tile_pool(name="dram", bufs=1, space="DRAM") as dpool,
            tc.tile_pool(name="psum", bufs=1, space="PSUM") as psum,
        ):
            # ---------------- constants ----------------
            ident = cpool.tile([128, 128], F32, name="ident")
            make_identity(nc, ident[:])
            ident_r = cpool.tile([128, 128], F32R, name="ident_r")
            nc.vector.tensor_copy(ident_r[:], ident[:])
            ones_col = cpool.tile([128, 1], BF16, name="ones_col")
            nc.gpsimd.memset(ones_col[:], 1.0)
            ones_row = cpool.tile([1, 128], F32, name="ones_row")
            nc.gpsimd.memset(ones_row[:], 1.0)
            ones_row_r = cpool.tile([1, 128], F32R, name="ones_row_r")
            nc.vector.tensor_copy(ones_row_r[:], ones_row[:])
            zsel = cpool.tile([128, 2], F32, name="zsel")
            nc.sync.dma_start(zsel[:], zsel_d[:])

            # persistent SBUF state
            qT_sb = [pers.tile([128, seq], BF16, name=f"qT{h}")
                     for h in range(HL)]
            kT_sb = [pers.tile([128, seq], BF16, name=f"kT{h}")
                     for h in range(HL)]
            v_sb = [pers.tile([128, QD], BF16, name=f"v{st}")
                    for st in range(ST)]
            bv_bc = pers.tile([128, QD], F32, name="bv_bc")

            # bias/mask tables (loaded on sync, after x chunk 0 and W; the
            # scalar/ACT queue stays clear of DMA-trigger instructions)
            bias_tables = [
                (pers.tile([128, S4 * ST], F32, name="bjv"), bjv_d),
                (pers.tile([128, 512], BF16, name="bim"), bim_d),
                (pers.tile([128, 4 * 512], BF16, name="bmask"), bmask_d),
                (pers.tile([128, 3 * S4 * ST], F32, name="bjv2"), bjv2_d),
                (pers.tile([128, 4 * 512], BF16, name="cau"), cau_d),
            ]
            bjv, bim, bmask, bjv2, cau = (t for t, _ in bias_tables)

            a2a_in = [dpool.tile([N_CORES * 128, RQ], BF16, name=f"a2ai{h}")
                      for h in range(HL)]
            a2a_out = [dpool.tile([N_CORES * 128, RQ], BF16, name=f"a2ao{h}")
                       for h in range(HL)]

            # ---------------- Phase A: QKV projection ----------------
            with (
                tc.tile_pool(name="wp", bufs=1) as wp,
                tc.tile_pool(name="xp", bufs=4) as xp,
                tc.tile_pool(name="xtp", bufs=2) as xtp,
            ):
                # bv broadcast row: borrow an xp ring slot transiently
                # (a [1,N] tile pads to 128 partitions, so a dedicated pool
                # would waste SBUF).
                brow = xp.tile([1, E], F32R, tag="xn", name="brow")
                nc.sync.dma_start(brow[:, :QD],
                                  _r(bv_d.rearrange("(o q) -> o q", o=1)))
                ps_bv = psum.tile([128, 512], F32, tag="bc", bufs=1,
                                  name="ps_bv")
                nc.tensor.matmul(ps_bv[:], ones_row_r[:],
                                 brow[:, :QD], start=True, stop=True)
                nc.scalar.copy(bv_bc[:], ps_bv[:])
                xtiles = {}

                def load_x(s4):
                    for st in range(4):
                        t = xp.tile([128, E], F32R, tag="xn", name="xn")
                        nc.sync.dma_start(
                            t[:], _r(x_d[(s4 * 4 + st) * 128:
                                         (s4 * 4 + st + 1) * 128, :]))
                        xtiles[(s4, st)] = t

                load_x(0)
                wt = {}
                for wi, wd in enumerate((wq_d, wk_d, wv_d)):
                    for et in range(ET):
                        t = wp.tile([128, QD], BF16, name=f"w{wi}_{et}")
                        nc.sync.dma_start(
                            t[:], wd[et * 128:(et + 1) * 128, :])
                        wt[(wi, et)] = t
                bvec = {}
                for bi, bd in enumerate((bq_d, bk_d)):
                    for m in range(HL):
                        t = cpool.tile([128, 1], F32, name=f"b{bi}_{m}")
                        nc.sync.dma_start(
                            t[:], bd[m * 128:(m + 1) * 128].rearrange(
                                "(p o) -> p o", o=1))
                        bvec[(bi, m)] = t
                for t, td in bias_tables:
                    nc.sync.dma_start(t[:], td[:])

                xT = {}

                def emit_transpose(s4, et):
                    # 4 transposes of [128,128] into one psum tile's quarters,
                    # then a single DVE copy casting to bf16.
                    pt = psum.tile([128, 512], F32R, tag="o", bufs=2,
                                   name="ps_tp")
                    for st in range(4):
                        nc.tensor.transpose(
                            pt[:, st * 128:(st + 1) * 128],
                            xtiles[(s4, st)][:, et * 128:(et + 1) * 128],
                            ident_r[:])
                    t = xtp.tile([128, 512], BF16, tag=f"xT{et}",
                                 name=f"xT{et}")
                    nc.vector.tensor_copy(t[:], pt[:].bitcast(F32))
                    xT[(s4, et)] = t

                for et in range(ET):
                    emit_transpose(0, et)

                for s4 in range(S4):
                    if s4 + 1 < S4:
                        load_x(s4 + 1)
                    tp_next = list(range(ET)) if s4 + 1 < S4 else []

                    def chain_qk(wi, m):
                        ps = psum.tile([128, 512], F32, tag="mm", bufs=4,
                                       name="ps_mm")
                        for et in range(ET):
                            nc.tensor.matmul(
                                ps[:],
                                wt[(wi, et)][:, m * 128:(m + 1) * 128],
                                xT[(s4, et)][:],
                                start=(et == 0), stop=(et == ET - 1))
                        dst = qT_sb[m] if wi == 0 else kT_sb[m]
                        nc.scalar.activation(
                            dst[:, s4 * 512:(s4 + 1) * 512], ps[:],
                            AF.Identity, bias=bvec[(wi, m)][:], scale=1.0)

                    def chain_v(st):
                        ps = psum.tile([128, 512], F32, tag="mm", bufs=4,
                                       name="ps_mv")
                        for et in range(ET):
                            nc.tensor.matmul(
                                ps[:],
                                xT[(s4, et)][:, st * 128:(st + 1) * 128],
                                wt[(2, et)][:],
                                start=(et == 0), stop=(et == ET - 1))
                        nc.vector.scalar_tensor_tensor(
                            v_sb[s4 * 4 + st][:], ps[:], 0.0, bv_bc[:],
                            AL.bypass, AL.add)

                    ci = 0
                    for wi in (0, 1):
                        for m in range(HL):
                            chain_qk(wi, m)
                            while len(tp_next) > (11 - ci) * ET // 12:
                                emit_transpose(s4 + 1, tp_next.pop(0))
                            ci += 1
                    for st in range(4):
                        chain_v(st)
                        while len(tp_next) > (11 - ci) * ET // 12:
                            emit_transpose(s4 + 1, tp_next.pop(0))
                        ci += 1

            # ---------------- Phase B: attention ----------------
            with (
                tc.tile_pool(name="wop", bufs=1) as wop,
                tc.tile_pool(name="hidp", bufs=1) as hidp,
                tc.tile_pool(name="bop", bufs=1) as bop,
                tc.tile_pool(name="pp", bufs=6) as ppool,
                tc.tile_pool(name="stgB", bufs=3) as stgB,
                tc.tile_pool(name="aop", bufs=3) as aop,
                tc.tile_pool(name="ldp", bufs=4) as ldp,
                tc.tile_pool(name="blt", bufs=2) as blt,
            ):
                hid = [hidp.tile([128, RQ], BF16, name=f"hid{k}")
                       for k in range(4 * HL)]
                bo_bc = bop.tile([128, E], F32, name="bo_bc")
                bo_row = bop.tile([1, E], F32R, name="bo_row")
                nc.sync.dma_start(bo_row[:],
                                  _r(bo_d.rearrange("(o q) -> o q", o=1)))
                for ct in range(4):
                    ps_bo = psum.tile([128, 512], F32, tag="bc", bufs=1,
                                      name="ps_bo")
                    nc.tensor.matmul(
                        ps_bo[:], ones_row_r[:],
                        bo_row[:, ct * 512:(ct + 1) * 512],
                        start=True, stop=True)
                    nc.scalar.copy(bo_bc[:, ct * 512:(ct + 1) * 512],
                                   ps_bo[:])
                # Wo half-0 prefetch (cols 0:1024), used by phase C.
                wo0 = []
                for k in range(4 * HL):
                    hl_, src_ = k // 4, k % 4
                    eg = head_of(src_, hl_) * 128
                    t = wop.tile([128, 1024], BF16, tag="wo", name="wok",
                                 bufs=16)
                    nc.sync.dma_start(t[:], wo_d[eg:eg + 128, 0:1024])
                    wo0.append(t)

                pending_cc = [None]

                def emit_cc(hl):
                    nc.gpsimd.collective_compute(
                        "AllToAll", AL.bypass,
                        replica_groups=[list(range(N_CORES))],
                        ins=[a2a_in[hl].opt()],
                        outs=[a2a_out[hl].opt()])

                def emit_blend(hl):
                    # Deferred to after all attention: a blend op in the
                    # middle of the DVE/sync queues would head-of-line
                    # block them on the AllToAll completion.
                    for src_ in range(4):
                        k = hl * 4 + src_
                        la = ldp.tile([128, RQ], BF16, tag="la", name="la")
                        nc.sync.dma_start(
                            la[:],
                            a2a_out[hl][src_ * 128:(src_ + 1) * 128, :])
                        lb = ldp.tile([128, RQ], BF16, tag="lb", name="lb")
                        nc.sync.dma_start(
                            lb[:],
                            a2a_out[hl][(src_ + 4) * 128:(src_ + 5) * 128, :])
                        tmp = blt.tile([128, RQ], BF16, tag="tmp", name="tmp")
                        nc.vector.tensor_scalar(
                            tmp[:], lb[:], zsel[:, 1:2], None, AL.mult)
                        nc.vector.scalar_tensor_tensor(
                            hid[k][:], la[:], zsel[:, 0:1], tmp[:],
                            AL.mult, AL.add)

                for hl in range(HL):
                    for im in range(S4):
                        njt = 4 * im + 4
                        kept = [jt for jt in range(njt)
                                if jt >= 4 * im - B_SLOT[hl]]
                        first, last = kept[0], kept[-1]
                        ps_o = psum.tile([128, 512], F32, tag="o", bufs=2,
                                         name="ps_o")
                        ps_d = psum.tile([1, 512], F32, tag="d", bufs=1,
                                         name="ps_d")

                        def consume(jt_, p_):
                            nc.tensor.matmul(ps_d[:], ones_col[:], p_[:],
                                             start=(jt_ == first),
                                             stop=(jt_ == last))
                            nc.tensor.matmul(
                                ps_o[:],
                                v_sb[jt_][:, hl * 128:(hl + 1) * 128], p_[:],
                                start=(jt_ == first), stop=(jt_ == last))

                        pipe = deque()
                        for ji, jt in enumerate(kept):
                            ps_s = psum.tile([128, 512], F32, tag="mm",
                                             bufs=4, name="ps_s")
                            nc.tensor.matmul(
                                ps_s[:],
                                kT_sb[hl][:, jt * 128:(jt + 1) * 128],
                                qT_sb[hl][:, im * 512:(im + 1) * 512],
                                start=True, stop=True)
                            r = jt - 4 * im
                            p = ppool.tile([128, 512], BF16, tag="p",
                                           name="p")
                            if hl < 3:
                                # factorized: exp(scale*s + sl*(j-anchor));
                                # the per-column exp(-sl*(i-anchor)) factor
                                # cancels in the normalization, so only the
                                # 0/1 causal pattern is applied on diagonal
                                # tiles.
                                idx = (hl * S4 + im) * ST + jt
                                jv2 = bjv2[:, idx:idx + 1]
                                nc.scalar.activation(p[:], ps_s[:], AF.Exp,
                                                     bias=jv2,
                                                     scale=SCALE)
                                if r >= 0:
                                    nc.vector.tensor_tensor(
                                        p[:], p[:],
                                        cau[:, r * 512:(r + 1) * 512],
                                        AL.mult)
                            else:
                                # largest slopes: pre-exp bias+mask add
                                # (DVE, PSUM) then plain exp.
                                idx = im * ST + jt
                                jv = bjv[:, idx:idx + 1]
                                if r >= 0:
                                    in1 = bmask[:, r * 512:(r + 1) * 512]
                                else:
                                    in1 = bim[:, :512]
                                nc.vector.scalar_tensor_tensor(
                                    ps_s[:], ps_s[:], jv, in1,
                                    AL.add, AL.add)
                                nc.scalar.activation(p[:], ps_s[:], AF.Exp,
                                                     scale=SCALE)
                            pipe.append((jt, p))
                            if len(pipe) > 3:
                                consume(*pipe.popleft())
                        while pipe:
                            consume(*pipe.popleft())

                        sr1 = stgB.tile([1, 512], F32, tag="sd", name="sr1")
                        nc.vector.reciprocal_approx_fast(sr1[:], ps_d[:])
                        srb = stgB.tile([128, 512], F32, tag="sr",
                                        name="srb")
                        nc.gpsimd.partition_broadcast(srb[:], sr1[:])
                        ao = aop.tile([128, 512], BF16, tag="ao", name="ao")
                        nc.vector.scalar_tensor_tensor(
                            ao[:], ps_o[:], 0.0, srb[:], AL.bypass, AL.mult)
                        for dup in (0, 4):
                            nc.sync.dma_start(
                                a2a_in[hl][(im + dup) * 128:
                                           (im + dup + 1) * 128, :],
                                ao[:])
                        if im == 0 and pending_cc[0] is not None:
                            emit_cc(pending_cc[0])
                            pending_cc[0] = None
                    pending_cc[0] = hl
                emit_cc(pending_cc[0])
                for hl in range(HL):
                    emit_blend(hl)

                # -------------- Phase C: output projection --------------
                with tc.tile_pool(name="stgC", bufs=4) as stgC:
                    for half in range(2):
                        if half == 0:
                            wo_tiles = wo0
                        else:
                            wo_tiles = []
                            for k in range(4 * HL):
                                hl_, src_ = k // 4, k % 4
                                eg = head_of(src_, hl_) * 128
                                t = wop.tile([128, 1024], BF16, tag="wo",
                                             name="wok2", bufs=16)
                                nc.sync.dma_start(
                                    t[:], wo_d[eg:eg + 128, 1024:2048])
                                wo_tiles.append(t)
                        # output-tile-major: stores spread through the half
                        # instead of clustering at its end
                        for rt in range(RT):
                            for cth in range(2):
                                ps = psum.tile([128, 512], F32, tag="mm",
                                               bufs=4, name="ps_c")
                                for k in range(4 * HL):
                                    nc.tensor.matmul(
                                        ps[:],
                                        hid[k][:, rt * 128:(rt + 1) * 128],
                                        wo_tiles[k][:, cth * 512:
                                                    (cth + 1) * 512],
                                        start=(k == 0), stop=(k == 4 * HL - 1))
                                ct = half * 2 + cth
                                so = stgC.tile([128, 512], F32, tag="soC",
                                               name="soC")
                                nc.vector.scalar_tensor_tensor(
                                    so[:], ps[:], 0.0,
                                    bo_bc[:, ct * 512:(ct + 1) * 512],
                                    AL.bypass, AL.add)
                                nc.sync.dma_start(
                                    out_d[rt * 128:(rt + 1) * 128,
                                          ct * 512:(ct + 1) * 512], so[:])

    nc.compile()
    return nc


def make_in_maps(x, Wqkv, bqkv, Wo, bo, seq=SEQ):
    import ml_dtypes
    x = np.asarray(x, np.float32)
    Wqkv = np.asarray(Wqkv, np.float32)
    bqkv = np.asarray(bqkv, np.float32)
    Wo = np.ascontiguousarray(
        np.asarray(Wo, np.float32).astype(ml_dtypes.bfloat16))
    bo = np.asarray(bo, np.float32)
    E = HIDDEN
    slopes = _slopes()
    jp = np.arange(128, dtype=np.float32)
    iif = np.arange(512, dtype=np.float32)
    bf16 = ml_dtypes.bfloat16
    in_maps = []
    for c in range(N_CORES):
        b, g = c // 4, c % 4
        heads = [head_of(g, hl) for hl in range(HL)]
        hcols = np.concatenate(
            [np.arange(h * HEAD, (h + 1) * HEAD) for h in heads])
        bjv = np.zeros((128, S4 * ST), np.float32)
        bim = np.zeros((128, 512), np.float32)
        bmask = np.zeros((128, 4 * 512), np.float32)
        bjv2 = np.zeros((128, 3 * S4 * ST), np.float32)
        cau = np.zeros((128, 4 * 512), np.float32)
        for r in range(4):
            cau[:, r * 512:(r + 1) * 512] = (
                iif[None, :] >= (128 * r + jp[:, None])).astype(np.float32)
        for hl in range(HL):
            sl_pre = slopes[heads[hl]] / SCALE   # pre-scale units
            sl = slopes[heads[hl]]               # post-scale units
            if hl < 3:
                # factorized path: column factor dropped (cancels in the
                # softmax normalization). Slot 2's larger slopes need a
                # mid-block anchor to keep the exponent in fp32 range.
                anchor = 256 if hl == 2 else 0
                for im in range(S4):
                    for jt in range(ST):
                        bjv2[:, (hl * S4 + im) * ST + jt] = sl * (
                            jp + 128 * jt - 512 * im - anchor)
            else:
                for im in range(S4):
                    for jt in range(ST):
                        bjv[:, im * ST + jt] = sl_pre * (
                            jt * 128 + jp - im * 512)
                bim[:, :] = -sl_pre * iif[None, :]
                for r in range(4):
                    blk = bmask[:, r * 512:(r + 1) * 512]
                    blk[:] = -sl_pre * iif[None, :]
                    keep = iif[None, :] >= (128 * r + jp[:, None])
                    blk[~keep] = NEG
        zsel = np.zeros((128, 2), np.float32)
        zsel[:, 0] = 1.0 if b == 0 else 0.0
        zsel[:, 1] = 1.0 - zsel[:, 0]
        castw = lambda a: np.ascontiguousarray(a.astype(bf16))
        in_maps.append({
            "x": np.ascontiguousarray(x[b, :seq]),
            "wq": castw(Wqkv[:, hcols]),
            "wk": castw(Wqkv[:, E + hcols]),
            "wv": castw(Wqkv[:, 2 * E + hcols]),
            "bq": np.ascontiguousarray(bqkv[hcols]),
            "bk": np.ascontiguousarray(bqkv[E + hcols]),
            "bv": np.ascontiguousarray(bqkv[2 * E + hcols]),
            "wo": Wo,
            "bo": bo.copy(),
            "bjv": bjv,
            "bim": np.ascontiguousarray(bim.astype(bf16)),
            "bmask": np.ascontiguousarray(bmask.astype(bf16)),
            "bjv2": bjv2,
            "cau": np.ascontiguousarray(cau.astype(bf16)),
            "zsel": zsel,
        })
    return in_maps


def unshard(outs, seq=SEQ):
    full = np.zeros((BATCH, seq, HIDDEN), np.float32)
    q = seq // 4
    for c in range(N_CORES):
        b, g = c // 4, c % 4
        full[b, g * q:(g + 1) * q, :] = outs[c]["out"]
    return full


_NC_CACHE = {}


def kernel(x, Wqkv, bqkv, Wo, bo):
    key = ("full", SEQ)
    if key not in _NC_CACHE:
        _NC_CACHE[key] = build_nc(SEQ)
    nc = _NC_CACHE[key]
    in_maps = make_in_maps(x, Wqkv, bqkv, Wo, bo)
    res = run_bass_kernel_spmd(nc, in_maps, core_ids=list(range(N_CORES)))
    return unshard(res.results)


# revision 42
# speedup vs baseline: 1.0416x; 1.0416x over previous
"""ALiBi causal attention block (QKV proj + attention + out proj) on 8 TRN2
NeuronCores, written in Bass/Tile. v3: all-SBUF, all-bf16, ALiBi tile skipping.

Sharding: batch(2) x head-group(4) -> 8 cores. Core c (b=c//4, g=c%4) runs
heads {(3-hl)*4+g : hl in 0..3} of its batch: slot hl on every core holds a
head from the same slope quartile, so the causal+ALiBi tile-skip pattern is
identical across cores (one SPMD program) and balanced. QKV projection and
attention are comm-free per core. A per-head 8-core AllToAll (bf16)
redistributes attention outputs from head-sharding to row-sharding; a zsel
0/1 blend drops the duplicate cross-batch shards. Each core then multiplies
its 512 output rows by the full Wo (bf16) and writes rows
[512g, 512(g+1)) of its batch.

v3 vs the DRAM-roundtrip baseline:
- q/k/v stay SBUF-resident between projection and attention (bf16); no
  per-head DRAM reloads in phase B.
- All GEMMs in bf16 (1 cycle/row on the PE); x transposed in fp32r and cast
  to bf16 on the PSUM->SBUF copy (ACT).
- ALiBi decays by ~e^-25 within B_SLOT tiles of the diagonal, so far
  below-diagonal score tiles are skipped: 121 of 160 tiles per core.
- Bias+mask tiles precomputed on host (bf16), DMA'd in; the gpsimd queue
  holds only collective triggers + half the bias-add STTs, so each head's
  AllToAll fires right after its outputs ship.
- Phase-B bias adds alternate DVE/Pool; exps on ACT; the PE stream is
  software-pipelined 2 deep so it never waits on the softmax chain.
- Transposes for chunk s4+1 emitted interleaved between chunk s4's
  projection chains.
"""

import math
from collections import deque

import numpy as np

import concourse.bass as bass
import concourse.mybir as mybir
import concourse.tile as tile
from concourse import bacc
from concourse.bass_utils import run_bass_kernel_spmd
from concourse.masks import make_identity

F32 = mybir.dt.float32
F32R = mybir.dt.float32r
BF16 = mybir.dt.bfloat16
AL = mybir.AluOpType
AF = mybir.ActivationFunctionType

HIDDEN = 2048
NUM_HEADS = 16
HEAD = 128
SEQ = 2048
BATCH = 2
N_CORES = 8
HL = 4                      # heads per core
QD = HL * HEAD              # 512 projected cols per core
SCALE = 1.0 / math.sqrt(HEAD)
NEG = -1.0e6
ST = SEQ // 128             # 16 seq tiles
S4 = SEQ // 512             # 4 coarse chunks
ET = HIDDEN // 128          # 16 contraction tiles
RQ = SEQ // 4               # 512 output rows per core
RT = RQ // 128              # 4

# Below-diagonal tiles kept per slot: keep jt >= 4*im - B_SLOT[hl]. Slot hl
# holds head (3-hl)*4+g, so slot 0 has the smallest slopes (keep everything)
# and slot 3 the largest (keep only 1 below-diagonal tile).
B_SLOT = [16, 9, 3, 1]


def head_of(g, hl):
    return (3 - hl) * 4 + g


def _slopes():
    if NUM_HEADS <= 8:
        return [1.0 / 2 ** k for k in range(NUM_HEADS)]
    return [1.0 / 2 ** (k / 2) for k in range(NUM_HEADS)]


def _r(ap):
    return ap.bitcast(F32R)


def build_nc(seq=SEQ):
    E = HIDDEN

    nc = bacc.Bacc("TRN2", target_bir_lowering=False, debug=False,
                   num_devices=N_CORES)

    x_d = nc.dram_tensor("x", [seq, E], F32, kind="ExternalInput").ap()
    wq_d = nc.dram_tensor("wq", [E, QD], BF16, kind="ExternalInput").ap()
    wk_d = nc.dram_tensor("wk", [E, QD], BF16, kind="ExternalInput").ap()
    wv_d = nc.dram_tensor("wv", [E, QD], BF16, kind="ExternalInput").ap()
    bq_d = nc.dram_tensor("bq", [QD], F32, kind="ExternalInput").ap()
    bk_d = nc.dram_tensor("bk", [QD], F32, kind="ExternalInput").ap()
    bv_d = nc.dram_tensor("bv", [QD], F32, kind="ExternalInput").ap()
    wo_d = nc.dram_tensor("wo", [E, E], BF16, kind="ExternalInput").ap()
    bo_d = nc.dram_tensor("bo", [E], F32, kind="ExternalInput").ap()
    # slot 3 (largest slopes): pre-exp bias tables (STT path)
    bjv_d = nc.dram_tensor("bjv", [128, S4 * ST], F32,
                           kind="ExternalInput").ap()
    bim_d = nc.dram_tensor("bim", [128, 512], BF16,
                           kind="ExternalInput").ap()
    bmask_d = nc.dram_tensor("bmask", [128, 4 * 512], BF16,
                             kind="ExternalInput").ap()
    # slots 0-2: factorized form. The per-column factor exp(-sl*(i-anchor))
    # cancels in the softmax normalization, so only the 0/1 causal pattern
    # (head-independent) is applied post-exp.
    bjv2_d = nc.dram_tensor("bjv2", [128, 3 * S4 * ST], F32,
                            kind="ExternalInput").ap()
    cau_d = nc.dram_tensor("cau", [128, 4 * 512], BF16,
                           kind="ExternalInput").ap()
    zsel_d = nc.dram_tensor("zsel", [128, 2], F32, kind="ExternalInput").ap()
    out_d = nc.dram_tensor("out", [RQ, E], F32, kind="ExternalOutput").ap()

    with tile.TileContext(nc) as tc:
        with (
            tc.tile_pool(name="const", bufs=1) as cpool,
            tc.tile_pool(name="persist", bufs=1) as pers,
            tc.tile_pool(name="dram", bufs=1, space="DRAM") as dpool,
            tc.tile_pool(name="psum", bufs=1, space="PSUM") as psum,
        ):
            # ---------------- constants ----------------
            ident = cpool.tile([128, 128], F32, name="ident")
            make_identity(nc, ident[:])
            ident_r = cpool.tile([128, 128], F32R, name="ident_r")
            nc.vector.tensor_copy(ident_r[:], ident[:])
            ones_col = cpool.tile([128, 1], BF16, name="ones_col")
            nc.gpsimd.memset(ones_col[:], 1.0)
            ones_row = cpool.tile([1, 128], F32, name="ones_row")
            nc.gpsimd.memset(ones_row[:], 1.0)
            ones_row_r = cpool.tile([1, 128], F32R, name="ones_row_r")
            nc.vector.tensor_copy(ones_row_r[:], ones_row[:])
            zsel = cpool.tile([128, 2], F32, name="zsel")
            nc.sync.dma_start(zsel[:], zsel_d[:])

            # persistent SBUF state
            qT_sb = [pers.tile([128, seq], BF16, name=f"qT{h}")
                     for h in range(HL)]
            kT_sb = [pers.tile([128, seq], BF16, name=f"kT{h}")
                     for h in range(HL)]
            v_sb = [pers.tile([128, QD], BF16, name=f"v{st}")
                    for st in range(ST)]
            bv_bc = pers.tile([128, QD], F32, name="bv_bc")

            # bias/mask tables (loaded on sync, after x chunk 0 and W; the
            # scalar/ACT queue stays clear of DMA-trigger instructions)
            bias_tables = [
                (pers.tile([128, S4 * ST], F32, name="bjv"), bjv_d),
                (pers.tile([128, 512], BF16, name="bim"), bim_d),
                (pers.tile([128, 4 * 512], BF16, name="bmask"), bmask_d),
                (pers.tile([128, 3 * S4 * ST], F32, name="bjv2"), bjv2_d),
                (pers.tile([128, 4 * 512], BF16, name="cau"), cau_d),
            ]
            bjv, bim, bmask, bjv2, cau = (t for t, _ in bias_tables)

            a2a_in = [dpool.tile([N_CORES * 128, RQ], BF16, name=f"a2ai{h}")
                      for h in range(HL)]
            a2a_out = [dpool.tile([N_CORES * 128, RQ], BF16, name=f"a2ao{h}")
                       for h in range(HL)]

            # ---------------- Phase A: QKV projection ----------------
            with (
                tc.tile_pool(name="wp", bufs=1) as wp,
                tc.tile_pool(name="xp", bufs=4) as xp,
                tc.tile_pool(name="xtp", bufs=2) as xtp,
            ):
                # bv broadcast row: borrow an xp ring slot transiently
                # (a [1,N] tile pads to 128 partitions, so a dedicated pool
                # would waste SBUF).
                brow = xp.tile([1, E], F32R, tag="xn", name="brow")
                nc.sync.dma_start(brow[:, :QD],
                                  _r(bv_d.rearrange("(o q) -> o q", o=1)))
                ps_bv = psum.tile([128, 512], F32, tag="bc", bufs=1,
                                  name="ps_bv")
                nc.tensor.matmul(ps_bv[:], ones_row_r[:],
                                 brow[:, :QD], start=True, stop=True)
                nc.scalar.copy(bv_bc[:], ps_bv[:])
                xtiles = {}

                def load_x(s4):
                    for st in range(4):
                        t = xp.tile([128, E], F32R, tag="xn", name="xn")
                        nc.sync.dma_start(
                            t[:], _r(x_d[(s4 * 4 + st) * 128:
                                         (s4 * 4 + st + 1) * 128, :]))
                        xtiles[(s4, st)] = t

                load_x(0)
                wt = {}
                for wi, wd in enumerate((wq_d, wk_d, wv_d)):
                    for et in range(ET):
                        t = wp.tile([128, QD], BF16, name=f"w{wi}_{et}")
                        nc.sync.dma_start(
                            t[:], wd[et * 128:(et + 1) * 128, :])
                        wt[(wi, et)] = t
                bvec = {}
                for bi, bd in enumerate((bq_d, bk_d)):
                    for m in range(HL):
                        t = cpool.tile([128, 1], F32, name=f"b{bi}_{m}")
                        nc.sync.dma_start(
                            t[:], bd[m * 128:(m + 1) * 128].rearrange(
                                "(p o) -> p o", o=1))
                        bvec[(bi, m)] = t
                for t, td in bias_tables:
                    nc.sync.dma_start(t[:], td[:])

                xT = {}

                def emit_transpose(s4, et):
                    # 4 transposes of [128,128] into one psum tile's quarters,
                    # then a single DVE copy casting to bf16.
                    pt = psum.tile([128, 512], F32R, tag="o", bufs=2,
                                   name="ps_tp")
                    for st in range(4):
                        nc.tensor.transpose(
                            pt[:, st * 128:(st + 1) * 128],
                            xtiles[(s4, st)][:, et * 128:(et + 1) * 128],
                            ident_r[:])
                    t = xtp.tile([128, 512], BF16, tag=f"xT{et}",
                                 name=f"xT{et}")
                    nc.vector.tensor_copy(t[:], pt[:].bitcast(F32))
                    xT[(s4, et)] = t

                for et in range(ET):
                    emit_transpose(0, et)

                for s4 in range(S4):
                    if s4 + 1 < S4:
                        load_x(s4 + 1)
                    tp_next = list(range(ET)) if s4 + 1 < S4 else []

                    def chain_qk(wi, m):
                        ps = psum.tile([128, 512], F32, tag="mm", bufs=4,
                                       name="ps_mm")
                        for et in range(ET):
                            nc.tensor.matmul(
                                ps[:],
                                wt[(wi, et)][:, m * 128:(m + 1) * 128],
                                xT[(s4, et)][:],
                                start=(et == 0), stop=(et == ET - 1))
                        dst = qT_sb[m] if wi == 0 else kT_sb[m]
                        nc.scalar.activation(
                            dst[:, s4 * 512:(s4 + 1) * 512], ps[:],
                            AF.Identity, bias=bvec[(wi, m)][:], scale=1.0)

                    def chain_v(st):
                        ps = psum.tile([128, 512], F32, tag="mm", bufs=4,
                                       name="ps_mv")
                        for et in range(ET):
                            nc.tensor.matmul(
                                ps[:],
                                xT[(s4, et)][:, st * 128:(st + 1) * 128],
                                wt[(2, et)][:],
                                start=(et == 0), stop=(et == ET - 1))
                        nc.vector.scalar_tensor_tensor(
                            v_sb[s4 * 4 + st][:], ps[:], 0.0, bv_bc[:],
                            AL.bypass, AL.add)

                    ci = 0
                    for wi in (0, 1):
                        for m in range(HL):
                            chain_qk(wi, m)
                            while len(tp_next) > (11 - ci) * ET // 12:
                                emit_transpose(s4 + 1, tp_next.pop(0))
                            ci += 1
                    for st in range(4):
                        chain_v(st)
                        while len(tp_next) > (11 - ci) * ET // 12:
                            emit_transpose(s4 + 1, tp_next.pop(0))
                        ci += 1

            # ---------------- Phase B: attention ----------------
            with (
                tc.tile_pool(name="wop", bufs=1) as wop,
                tc.tile_pool(name="hidp", bufs=1) as hidp,
                tc.tile_pool(name="bop", bufs=1) as bop,
                tc.tile_pool(name="pp", bufs=6) as ppool,
                tc.tile_pool(name="stgB", bufs=3) as stgB,
                tc.tile_pool(name="aop", bufs=3) as aop,
                tc.tile_pool(name="ldp", bufs=4) as ldp,
                tc.tile_pool(name="blt", bufs=2) as blt,
            ):
                hid = [hidp.tile([128, RQ], BF16, name=f"hid{k}")
                       for k in range(4 * HL)]
                bo_bc = bop.tile([128, E], F32, name="bo_bc")
                bo_row = bop.tile([1, E], F32R, name="bo_row")
                nc.sync.dma_start(bo_row[:],
                                  _r(bo_d.rearrange("(o q) -> o q", o=1)))
                for ct in range(4):
                    ps_bo = psum.tile([128, 512], F32, tag="bc", bufs=1,
                                      name="ps_bo")
                    nc.tensor.matmul(
                        ps_bo[:], ones_row_r[:],
                        bo_row[:, ct * 512:(ct + 1) * 512],
                        start=True, stop=True)
                    nc.scalar.copy(bo_bc[:, ct * 512:(ct + 1) * 512],
                                   ps_bo[:])
                # Wo half-0 prefetch (cols 0:1024), used by phase C.
                wo0 = []
                for k in range(4 * HL):
                    hl_, src_ = k // 4, k % 4
                    eg = head_of(src_, hl_) * 128
                    t = wop.tile([128, 1024], BF16, tag="wo", name="wok",
                                 bufs=16)
                    nc.sync.dma_start(t[:], wo_d[eg:eg + 128, 0:1024])
                    wo0.append(t)

                pending_cc = [None]

                def emit_cc(hl):
                    nc.gpsimd.collective_compute(
                        "AllToAll", AL.bypass,
                        replica_groups=[list(range(N_CORES))],
                        ins=[a2a_in[hl].opt()],
                        outs=[a2a_out[hl].opt()])

                def emit_blend(hl):
                    # Deferred to after all attention: a blend op in the
                    # middle of the DVE/sync queues would head-of-line
                    # block them on the AllToAll completion.
                    for src_ in range(4):
                        k = hl * 4 + src_
                        la = ldp.tile([128, RQ], BF16, tag="la", name="la")
                        nc.sync.dma_start(
                            la[:],
                            a2a_out[hl][src_ * 128:(src_ + 1) * 128, :])
                        lb = ldp.tile([128, RQ], BF16, tag="lb", name="lb")
                        nc.sync.dma_start(
                            lb[:],
                            a2a_out[hl][(src_ + 4) * 128:(src_ + 5) * 128, :])
                        tmp = blt.tile([128, RQ], BF16, tag="tmp", name="tmp")
                        nc.vector.tensor_scalar(
                            tmp[:], lb[:], zsel[:, 1:2], None, AL.mult)
                        nc.vector.scalar_tensor_tensor(
                            hid[k][:], la[:], zsel[:, 0:1], tmp[:],
                            AL.mult, AL.add)

                for hl in range(HL):
                    for im in range(S4):
                        njt = 4 * im + 4
                        kept = [jt for jt in range(njt)
                                if jt >= 4 * im - B_SLOT[hl]]
                        first, last = kept[0], kept[-1]
                        ps_o = psum.tile([128, 512], F32, tag="o", bufs=2,
                                         name="ps_o")
                        ps_d = psum.tile([1, 512], F32, tag="d", bufs=1,
                                         name="ps_d")

                        def consume(jt_, p_):
                            nc.tensor.matmul(ps_d[:], ones_col[:], p_[:],
                                             start=(jt_ == first),
                                             stop=(jt_ == last))
                            nc.tensor.matmul(
                                ps_o[:],
                                v_sb[jt_][:, hl * 128:(hl + 1) * 128], p_[:],
                                start=(jt_ == first), stop=(jt_ == last))

                        pipe = deque()
                        for ji, jt in enumerate(kept):
                            ps_s = psum.tile([128, 512], F32, tag="mm",
                                             bufs=4, name="ps_s")
                            nc.tensor.matmul(
                                ps_s[:],
                                kT_sb[hl][:, jt * 128:(jt + 1) * 128],
                                qT_sb[hl][:, im * 512:(im + 1) * 512],
                                start=True, stop=True)
                            r = jt - 4 * im
                            p = ppool.tile([128, 512], BF16, tag="p",
                                           name="p")
                            if hl < 3:
                                # factorized: exp(scale*s + sl*(j-anchor));
                                # the per-column exp(-sl*(i-anchor)) factor
                                # cancels in the normalization, so only the
                                # 0/1 causal pattern is applied on diagonal
                                # tiles.
                                idx = (hl * S4 + im) * ST + jt
                                jv2 = bjv2[:, idx:idx + 1]
                                nc.scalar.activation(p[:], ps_s[:], AF.Exp,
                                                     bias=jv2,
                                                     scale=SCALE)
                                if r >= 0:
                                    nc.vector.tensor_tensor(
                                        p[:], p[:],
                                        cau[:, r * 512:(r + 1) * 512],
                                        AL.mult)
                            else:
                                # largest slopes: pre-exp bias+mask add
                                # (DVE, PSUM) then plain exp.
                                idx = im * ST + jt
                                jv = bjv[:, idx:idx + 1]
                                if r >= 0:
                                    in1 = bmask[:, r * 512:(r + 1) * 512]
                                else:
                                    in1 = bim[:, :512]
                                nc.vector.scalar_tensor_tensor(
                                    ps_s[:], ps_s[:], jv, in1,
                                    AL.add, AL.add)
                                nc.scalar.activation(p[:], ps_s[:], AF.Exp,
                                                     scale=SCALE)
                            pipe.append((jt, p))
                            if len(pipe) > 3:
                                consume(*pipe.popleft())
                        while pipe:
                            consume(*pipe.popleft())

                        sr1 = stgB.tile([1, 512], F32, tag="sd", name="sr1")
                        nc.vector.reciprocal_approx_fast(sr1[:], ps_d[:])
                        srb = stgB.tile([128, 512], F32, tag="sr",
                                        name="srb")
                        nc.gpsimd.partition_broadcast(srb[:], sr1[:])
                        ao = aop.tile([128, 512], BF16, tag="ao", name="ao")
                        nc.vector.scalar_tensor_tensor(
                            ao[:], ps_o[:], 0.0, srb[:], AL.bypass, AL.mult)
                        for dup in (0, 4):
                            nc.sync.dma_start(
                                a2a_in[hl][(im + dup) * 128:
                                           (im + dup + 1) * 128, :],
                                ao[:])
                        if im == 0 and pending_cc[0] is not None:
                            emit_cc(pending_cc[0])
                            pending_cc[0] = None
                            # blend for the head whose A2A has long
                            # completed (two heads back): its ops never
                            # head-of-line block the DVE/sync queues.
                            if hl >= 2:
                                emit_blend(hl - 2)
                    pending_cc[0] = hl
                emit_cc(pending_cc[0])
                emit_blend(2)
                emit_blend(3)

                # -------------- Phase C: output projection --------------
                with tc.tile_pool(name="stgC", bufs=4) as stgC:
                    acc_spec = [("mm", 4), ("mm", 4), ("mm", 4), ("mm", 4),
                                ("o", 2), ("o", 2), ("d", 1), ("bc", 1)]
                    for half in range(2):
                        if half == 0:
                            wo_tiles = wo0
                        else:
                            wo_tiles = []
                            for k in range(4 * HL):
                                hl_, src_ = k // 4, k % 4
                                eg = head_of(src_, hl_) * 128
                                t = wop.tile([128, 1024], BF16, tag="wo",
                                             name="wok2", bufs=16)
                                nc.sync.dma_start(
                                    t[:], wo_d[eg:eg + 128, 1024:2048])
                                wo_tiles.append(t)
                        pos = [psum.tile([128, 512], F32, tag=tg, bufs=bf_,
                                         name="ps_c")
                               for tg, bf_ in acc_spec]
                        for k in range(4 * HL):
                            wt_ = wo_tiles[k]
                            for rt in range(RT):
                                for cth in range(2):
                                    nc.tensor.matmul(
                                        pos[rt * 2 + cth][:],
                                        hid[k][:, rt * 128:(rt + 1) * 128],
                                        wt_[:, cth * 512:(cth + 1) * 512],
                                        start=(k == 0), stop=(k == 4 * HL - 1))
                        for rt in range(RT):
                            for cth in range(2):
                                ct = half * 2 + cth
                                so = stgC.tile([128, 512], F32, tag="soC",
                                               name="soC")
                                nc.vector.scalar_tensor_tensor(
                                    so[:], pos[rt * 2 + cth][:], 0.0,
                                    bo_bc[:, ct * 512:(ct + 1) * 512],
                                    AL.bypass, AL.add)
                                nc.sync.dma_start(
                                    out_d[rt * 128:(rt + 1) * 128,
                                          ct * 512:(ct + 1) * 512], so[:])

    nc.compile()
    return nc


def make_in_maps(x, Wqkv, bqkv, Wo, bo, seq=SEQ):
    import ml_dtypes
    x = np.asarray(x, np.float32)
    Wqkv = np.asarray(Wqkv, np.float32)
    bqkv = np.asarray(bqkv, np.float32)
    Wo = np.ascontiguousarray(
        np.asarray(Wo, np.float32).astype(ml_dtypes.bfloat16))
    bo = np.asarray(bo, np.float32)
    E = HIDDEN
    slopes = _slopes()
    jp = np.arange(128, dtype=np.float32)
    iif = np.arange(512, dtype=np.float32)
    bf16 = ml_dtypes.bfloat16
    in_maps = []
    for c in range(N_CORES):
        b, g = c // 4, c % 4
        heads = [head_of(g, hl) for hl in range(HL)]
        hcols = np.concatenate(
            [np.arange(h * HEAD, (h + 1) * HEAD) for h in heads])
        bjv = np.zeros((128, S4 * ST), np.float32)
        bim = np.zeros((128, 512), np.float32)
        bmask = np.zeros((128, 4 * 512), np.float32)
        bjv2 = np.zeros((128, 3 * S4 * ST), np.float32)
        cau = np.zeros((128, 4 * 512), np.float32)
        for r in range(4):
            cau[:, r * 512:(r + 1) * 512] = (
                iif[None, :] >= (128 * r + jp[:, None])).astype(np.float32)
        for hl in range(HL):
            sl_pre = slopes[heads[hl]] / SCALE   # pre-scale units
            sl = slopes[heads[hl]]               # post-scale units
            if hl < 3:
                # factorized path: column factor dropped (cancels in the
                # softmax normalization). Slot 2's larger slopes need a
                # mid-block anchor to keep the exponent in fp32 range.
                anchor = 256 if hl == 2 else 0
                for im in range(S4):
                    for jt in range(ST):
                        bjv2[:, (hl * S4 + im) * ST + jt] = sl * (
                            jp + 128 * jt - 512 * im - anchor)
            else:
                for im in range(S4):
                    for jt in range(ST):
                        bjv[:, im * ST + jt] = sl_pre * (
                            jt * 128 + jp - im * 512)
                bim[:, :] = -sl_pre * iif[None, :]
                for r in range(4):
                    blk = bmask[:, r * 512:(r + 1) * 512]
                    blk[:] = -sl_pre * iif[None, :]
                    keep = iif[None, :] >= (128 * r + jp[:, None])
                    blk[~keep] = NEG
        zsel = np.zeros((128, 2), np.float32)
        zsel[:, 0] = 1.0 if b == 0 else 0.0
        zsel[:, 1] = 1.0 - zsel[:, 0]
        castw = lambda a: np.ascontiguousarray(a.astype(bf16))
        in_maps.append({
            "x": np.ascontiguousarray(x[b, :seq]),
            "wq": castw(Wqkv[:, hcols]),
            "wk": castw(Wqkv[:, E + hcols]),
            "wv": castw(Wqkv[:, 2 * E + hcols]),
            "bq": np.ascontiguousarray(bqkv[hcols]),
            "bk": np.ascontiguousarray(bqkv[E + hcols]),
            "bv": np.ascontiguousarray(bqkv[2 * E + hcols]),
            "wo": Wo,
            "bo": bo.copy(),
            "bjv": bjv,
            "bim": np.ascontiguousarray(bim.astype(bf16)),
            "bmask": np.ascontiguousarray(bmask.astype(bf16)),
            "bjv2": bjv2,
            "cau": np.ascontiguousarray(cau.astype(bf16)),
            "zsel": zsel,
        })
    return in_maps


def unshard(outs, seq=SEQ):
    full = np.zeros((BATCH, seq, HIDDEN), np.float32)
    q = seq // 4
    for c in range(N_CORES):
        b, g = c // 4, c % 4
        full[b, g * q:(g + 1) * q, :] = outs[c]["out"]
    return full


_NC_CACHE = {}


def kernel(x, Wqkv, bqkv, Wo, bo):
    key = ("full", SEQ)
    if key not in _NC_CACHE:
        _NC_CACHE[key] = build_nc(SEQ)
    nc = _NC_CACHE[key]
    in_maps = make_in_maps(x, Wqkv, bqkv, Wo, bo)
    res = run_bass_kernel_spmd(nc, in_maps, core_ids=list(range(N_CORES)))
    return unshard(res.results)
